# revision 10
# baseline (speedup 1.0000x reference)
"""Trainium2 Bass kernel for the ContinuousVariableQNN problem.

Math reduction (validated against the jax reference on host):
  The reference builds a 256x256 symplectic matrix S from params, then
    mu   = mu0 @ S.T   with mu0[:, 0::2] = 2*inputs (odd cols zero)
    n    = (dsum + mu_x^2 + mu_p^2) / (2*hbar) - 0.5
  Because mu0's p-quadrature entries are all zero, the big matmul collapses to
    mu_dev = inputs @ Ms          with Ms[i, j] = S[j, 2*i]   ([128, 256])
  (factor 2 from displacement and the 1/4 normalization cancel), and
    n[b, m] = mu_dev[b, 2m]^2 + mu_dev[b, 2m+1]^2 + bias[m]
  with bias[m] = (diag(S S^T)[2m] + diag(S S^T)[2m+1])/4 - 0.5 (a constant).

Device strategy (pure data parallelism over 8 cores, batch-sharded):
  The batch is transposed on the HOST so each core receives
  xt [128 features, 16384 batch] -- fully contiguous DMA, no on-chip
  transposes.  Everything runs in float16 on the PE: fp16 streams at
  1 cycle/row (vs fp32r which draws enough power to trip the 0.5-util
  EDPP throttle) and halves input DMA traffic.  fp16's 11 mantissa bits
  survive the ~12x error amplification of this problem (sims at 7.8e-3
  vs the 2e-2 gate; bf16 inputs sim at 2.4e-2 and fail).  Ms overflows
  fp16 range, so the host pre-scales it by a global power of two and
  folds s^2 into the final host-side bias add.

  Mode-stationary matmuls: mu_x.T [128 modes, 512] = Mse.T @ xt chunk,
  ditto mu_p with Mso, PSUM tiles of 1024 (2 banks, bufs=2 -> all 8).
  Squares: ACT Square straight from PSUM (engines may read only ONE
  operand from PSUM, so DVE cannot self-mult PSUM); for 10/16 sub-chunks
  the p-half goes Pool-copy -> DVE self-mult to keep ACT under the DMA
  envelope.  Pair-add on DVE in bf16 (2x mode).  n.T goes back as bf16
  (output traffic halved); per-mode bias lands on the host for free.
"""

import ml_dtypes
import numpy as np

import concourse.bass as bass
import concourse.mybir as mybir
import concourse.tile as tile
from concourse import bacc
from concourse.bass_utils import run_bass_kernel_spmd

N_QUMODES = 128
N_LAYERS = 8
BATCH = 131072
N_CORES = 8
ROWS = BATCH // N_CORES          # 16384 batch columns per core
CHUNK = 2048                     # batch columns per input DMA chunk
N_CHUNKS = ROWS // CHUNK         # 8
SUB = 1024                       # batch columns per compute sub-chunk
SUBS_PER_CHUNK = CHUNK // SUB    # 2
N_SUBS = N_CHUNKS * SUBS_PER_CHUNK
MM = 512                         # matmul free dim (one PSUM bank of fp32)
F32 = mybir.dt.float32
F16 = mybir.dt.float16
BF16 = mybir.dt.bfloat16

# fp16 scaling for Ms (entries up to ~3e5 overflow fp16's 65504 max).
MS_TARGET_MAX = 16384.0


def host_prep(params: np.ndarray):
    """Build fp16 ms [128, 256] = [Mse | Mso]/s, bias [128], and s^2."""
    L, N = N_LAYERS, N_QUMODES
    p = params.reshape(L, N, 3).astype(np.float64)
    th1, r, th2 = p[..., 0], p[..., 1], p[..., 2]

    def rot(th):
        c, s = np.cos(th), np.sin(th)
        return np.stack([np.stack([c, -s], -1), np.stack([s, c], -1)], -2)

    z = np.zeros_like(r)
    sq = np.stack([np.stack([np.exp(-r), z], -1),
                   np.stack([z, np.exp(r)], -1)], -2)
    blk = np.einsum('lnab,lnbc,lncd->lnad', rot(th2), sq, rot(th1))

    t = np.cos(np.pi / 4)
    rr = np.sin(np.pi / 4)
    BS4 = np.array([[t, 0., -rr, 0.],
                    [0., t, 0., -rr],
                    [rr, 0., t, 0.],
                    [0., rr, 0., t]])
    C = np.eye(2 * N)
    for i in range(N - 1):
        C[2 * i:2 * i + 4, :] = BS4 @ C[2 * i:2 * i + 4, :]

    S = np.eye(2 * N)
    idx = np.arange(N)
    for l in range(L):
        D = np.zeros((N, 2, N, 2))
        D[idx, :, idx, :] = blk[l]
        S = C @ (D.reshape(2 * N, 2 * N) @ S)

    # Ms[i, j] = S[j, 2i]; mode-stationary halves Mse[i,m]=Ms[i,2m],
    # Mso[i,m]=Ms[i,2m+1] packed side by side for one DMA.
    Ms = S[:, 0::2].T                                   # [128, 256]
    ms_cat = np.concatenate([Ms[:, 0::2], Ms[:, 1::2]], axis=1)

    s = 2.0 ** np.ceil(np.log2(np.abs(ms_cat).max() / MS_TARGET_MAX))
    s = max(s, 1.0)
    ms_f16 = np.ascontiguousarray(ms_cat / s).astype(np.float16)

    dV = (S ** 2).sum(axis=1)                           # [256]
    bias = ((dV[0::2] + dV[1::2]) / 4.0 - 0.5).astype(np.float32)  # [128]
    return ms_f16, bias, np.float32(s * s)


def make_in_maps(X: np.ndarray, ms_f16: np.ndarray):
    """Per-core input dicts: xt [128, ROWS] f16 (host-transposed), ms."""
    Xt = np.ascontiguousarray(
        X.reshape(N_CORES, ROWS, N_QUMODES).transpose(0, 2, 1).astype(np.float16))
    return [{"xt": Xt[i], "ms": ms_f16} for i in range(N_CORES)]


def postprocess(results, bias: np.ndarray, s2: np.float32) -> np.ndarray:
    """Gather per-core n.T bf16 tiles into the full [BATCH, 128] f32 output,
    undoing the fp16 weight scale and adding the per-mode bias."""
    out = np.empty((BATCH, N_QUMODES), dtype=np.float32)
    for i, r in enumerate(results):
        out[i * ROWS:(i + 1) * ROWS, :] = r["out"].astype(np.float32).T
    out *= s2
    out += bias[None, :]
    return out


def build_bass():
    nc = bacc.Bacc("TRN2", target_bir_lowering=False, debug=False,
                   num_devices=N_CORES)

    xt_d = nc.dram_tensor("xt", [128, ROWS], F16, kind="ExternalInput")
    ms_d = nc.dram_tensor("ms", [128, 256], F16, kind="ExternalInput")
    out_d = nc.dram_tensor("out", [128, ROWS], BF16, kind="ExternalOutput")

    xt_v = xt_d.ap()
    out_v = out_d.ap()

    # Per-sub p-square recipe: 'A' = ACT Square from PSUM + DVE add;
    # 'D' = DVE copy + DVE self-mult + Pool add;
    # 'P' = DVE copy + Pool self-mult + DVE add.
    # ACT is the only engine that can square straight from PSUM (one read);
    # DVE cannot read PSUM twice and Pool cannot read PSUM at all.  The
    # slow Pool ops are kept away from the last subs so the tail drains
    # through the short ACT/DVE chains.
    RECIPE = ['A', 'A', 'D', 'P', 'D', 'P', 'D', 'P', 'D', 'P', 'D', 'P',
              'A', 'A', 'A', 'A']
    N_WARMUP = 10                    # PE p-state warmup matmuls on zeros

    with tile.TileContext(nc) as tc:
        with (
            tc.tile_pool(name="const", bufs=1) as const_pool,
            tc.tile_pool(name="xin", bufs=3) as xin_pool,
            tc.tile_pool(name="oout", bufs=3) as oout_pool,
            tc.tile_pool(name="sqx", bufs=3) as sqx_pool,
            tc.tile_pool(name="sqp", bufs=3) as sqp_pool,
            tc.tile_pool(name="cp", bufs=2) as cp_pool,
            tc.tile_pool(name="mux", bufs=2, space="PSUM") as mux_pool,
            tc.tile_pool(name="mup", bufs=2, space="PSUM") as mup_pool,
        ):
            # Tiny ms halves first so the PE can start as soon as x lands.
            mse = const_pool.tile([128, 128], F16)
            nc.sync.dma_start(out=mse, in_=ms_d.ap()[:, 0:128])
            mso = const_pool.tile([128, 128], F16)
            nc.sync.dma_start(out=mso, in_=ms_d.ap()[:, 128:256])

            x_tiles: dict[int, bass.AP] = {}

            def load_chunk(c):
                x_sb = xin_pool.tile([128, CHUNK], F16, tag="x_sb",
                                     name=f"x_sb_{c}")
                if c == 0:
                    # split the first transfer so the PE can start sooner
                    for q in range(4):
                        nc.sync.dma_start(
                            out=x_sb[:, q * MM:(q + 1) * MM],
                            in_=xt_v[:, q * MM:(q + 1) * MM])
                else:
                    nc.sync.dma_start(out=x_sb,
                                      in_=xt_v[:, c * CHUNK:(c + 1) * CHUNK])
                x_tiles[c] = x_sb

            load_chunk(0)
            load_chunk(1)

            # Warm the PE out of its low p-state while the first input
            # chunk is still in flight: matmuls over zeroed scratch keep
            # the array busy; results are never read.
            warm = const_pool.tile([128, 128], F16)
            nc.gpsimd.memset(warm, 0)
            warm_ps = mux_pool.tile([128, SUB], F32, tag="mu")
            for _ in range(N_WARMUP):
                nc.tensor.matmul(warm_ps[:, 0:128], warm, warm,
                                 start=True, stop=True)

            o_tiles: dict[int, bass.AP] = {}
            for i in range(N_SUBS):
                c, sc = divmod(i, SUBS_PER_CHUNK)
                if sc == 0 and c + 2 < N_CHUNKS:
                    load_chunk(c + 2)
                x_sb = x_tiles[c]

                mu_x = mux_pool.tile([128, SUB], F32, tag="mu")  # 2 PSUM banks
                mu_p = mup_pool.tile([128, SUB], F32, tag="mp")  # 2 PSUM banks
                for q in range(SUB // MM):
                    rhs = x_sb[:, sc * SUB + q * MM: sc * SUB + (q + 1) * MM]
                    nc.tensor.matmul(mu_x[:, q * MM:(q + 1) * MM], mse, rhs,
                                     start=True, stop=True)
                    nc.tensor.matmul(mu_p[:, q * MM:(q + 1) * MM], mso, rhs,
                                     start=True, stop=True)

                sq_x = sqx_pool.tile([128, SUB], BF16, tag="sq_x",
                                     name=f"sq_x_{i}")
                sq_p = sqp_pool.tile([128, SUB], BF16, tag="sq_p",
                                     name=f"sq_p_{i}")
                nc.scalar.activation(sq_x, mu_x,
                                     mybir.ActivationFunctionType.Square)
                recipe = RECIPE[i]
                if recipe == 'A':
                    nc.scalar.activation(sq_p, mu_p,
                                         mybir.ActivationFunctionType.Square)
                    add_eng = nc.vector
                elif recipe == 'D':
                    cp = cp_pool.tile([128, SUB], F32, tag="cp",
                                      name=f"cp_{i}")
                    nc.vector.tensor_copy(cp, mu_p)
                    nc.vector.tensor_tensor(out=sq_p, in0=cp, in1=cp,
                                            op=mybir.AluOpType.mult)
                    add_eng = nc.gpsimd
                else:
                    cp = cp_pool.tile([128, SUB], F32, tag="cp",
                                      name=f"cp_{i}")
                    nc.vector.tensor_copy(cp, mu_p)
                    nc.gpsimd.tensor_tensor(out=sq_p, in0=cp, in1=cp,
                                            op=mybir.AluOpType.mult)
                    add_eng = nc.vector

                if sc == 0:
                    o_tiles[c] = oout_pool.tile([128, CHUNK], BF16, tag="o_sb",
                                                name=f"o_sb_{c}")
                o_sb = o_tiles[c]
                add_eng.tensor_tensor(out=o_sb[:, sc * SUB:(sc + 1) * SUB],
                                      in0=sq_x, in1=sq_p,
                                      op=mybir.AluOpType.add)
                # One output DMA per chunk, except the last chunk which
                # drains per-sub so the tail is as short as possible.
                if c == N_CHUNKS - 1:
                    nc.scalar.dma_start(
                        out=out_v[:, i * SUB:(i + 1) * SUB],
                        in_=o_sb[:, sc * SUB:(sc + 1) * SUB])
                elif sc == SUBS_PER_CHUNK - 1:
                    nc.scalar.dma_start(
                        out=out_v[:, c * CHUNK:(c + 1) * CHUNK],
                        in_=o_tiles.pop(c))
                if sc == SUBS_PER_CHUNK - 1:
                    x_tiles.pop(c, None)

    nc.compile()
    return nc


_NC_CACHE = None


def kernel(**inputs: np.ndarray) -> np.ndarray:
    global _NC_CACHE
    X = np.ascontiguousarray(np.asarray(inputs["inputs"], dtype=np.float32))
    params = np.asarray(inputs["params"], dtype=np.float32)
    assert X.shape == (BATCH, N_QUMODES)

    ms_f16, bias, s2 = host_prep(params)

    if _NC_CACHE is None:
        _NC_CACHE = build_bass()
    nc = _NC_CACHE

    in_maps = make_in_maps(X, ms_f16)
    res = run_bass_kernel_spmd(nc, in_maps, core_ids=list(range(N_CORES)))
    return postprocess(res.results, bias, s2)


# revision 11
# speedup vs baseline: 1.1895x; 1.1895x over previous
"""Trainium2 Bass kernel for the ContinuousVariableQNN problem.

Math reduction (validated against the jax reference on host):
  The reference builds a 256x256 symplectic matrix S from params, then
    mu   = mu0 @ S.T   with mu0[:, 0::2] = 2*inputs (odd cols zero)
    n    = (dsum + mu_x^2 + mu_p^2) / (2*hbar) - 0.5
  Because mu0's p-quadrature entries are all zero, the big matmul collapses to
    mu_dev = inputs @ Ms          with Ms[i, j] = S[j, 2*i]   ([128, 256])
  (factor 2 from displacement and the 1/4 normalization cancel), and
    n[b, m] = mu_dev[b, 2m]^2 + mu_dev[b, 2m+1]^2 + bias[m]
  with bias[m] = (diag(S S^T)[2m] + diag(S S^T)[2m+1])/4 - 0.5 (a constant).

Device strategy (pure data parallelism over 8 cores, batch-sharded):
  The batch is transposed on the HOST so each core receives
  xt [128 features, 16384 batch] -- fully contiguous DMA, no on-chip
  transposes.  Everything runs in float16 on the PE: fp16 streams at
  1 cycle/row (vs fp32r which draws enough power to trip the 0.5-util
  EDPP throttle) and halves input DMA traffic.  fp16's 11 mantissa bits
  survive the ~12x error amplification of this problem (sims at 7.8e-3
  vs the 2e-2 gate; bf16 inputs sim at 2.4e-2 and fail).  Ms overflows
  fp16 range, so the host pre-scales it by a global power of two and
  folds s^2 into the final host-side bias add.

  Mode-stationary matmuls: mu_x.T [128 modes, 512] = Mse.T @ xt chunk,
  ditto mu_p with Mso, PSUM tiles of 1024 (2 banks, bufs=2 -> all 8).
  Squares: ACT Square straight from PSUM (engines may read only ONE
  operand from PSUM, so DVE cannot self-mult PSUM); for 10/16 sub-chunks
  the p-half goes Pool-copy -> DVE self-mult to keep ACT under the DMA
  envelope.  Pair-add on DVE in bf16 (2x mode).  n.T goes back as bf16
  (output traffic halved); per-mode bias lands on the host for free.
"""

import ml_dtypes
import numpy as np

import concourse.bass as bass
import concourse.mybir as mybir
import concourse.tile as tile
from concourse import bacc
from concourse.bass_utils import run_bass_kernel_spmd

N_QUMODES = 128
N_LAYERS = 8
BATCH = 131072
N_CORES = 8
ROWS = BATCH // N_CORES          # 16384 batch columns per core
CHUNK = 2048                     # batch columns per input DMA chunk
N_CHUNKS = ROWS // CHUNK         # 8
SUB = 1024                       # batch columns per compute sub-chunk
SUBS_PER_CHUNK = CHUNK // SUB    # 2
N_SUBS = N_CHUNKS * SUBS_PER_CHUNK
MM = 512                         # matmul free dim (one PSUM bank of fp32)
F32 = mybir.dt.float32
F16 = mybir.dt.float16
BF16 = mybir.dt.bfloat16

# fp16 scaling for Ms (entries up to ~3e5 overflow fp16's 65504 max).
MS_TARGET_MAX = 16384.0


def host_prep(params: np.ndarray):
    """Build fp16 ms [128, 256] = [Mse | Mso]/s, bias [128], and s^2."""
    L, N = N_LAYERS, N_QUMODES
    p = params.reshape(L, N, 3).astype(np.float64)
    th1, r, th2 = p[..., 0], p[..., 1], p[..., 2]

    def rot(th):
        c, s = np.cos(th), np.sin(th)
        return np.stack([np.stack([c, -s], -1), np.stack([s, c], -1)], -2)

    z = np.zeros_like(r)
    sq = np.stack([np.stack([np.exp(-r), z], -1),
                   np.stack([z, np.exp(r)], -1)], -2)
    blk = np.einsum('lnab,lnbc,lncd->lnad', rot(th2), sq, rot(th1))

    t = np.cos(np.pi / 4)
    rr = np.sin(np.pi / 4)
    BS4 = np.array([[t, 0., -rr, 0.],
                    [0., t, 0., -rr],
                    [rr, 0., t, 0.],
                    [0., rr, 0., t]])
    C = np.eye(2 * N)
    for i in range(N - 1):
        C[2 * i:2 * i + 4, :] = BS4 @ C[2 * i:2 * i + 4, :]

    S = np.eye(2 * N)
    idx = np.arange(N)
    for l in range(L):
        D = np.zeros((N, 2, N, 2))
        D[idx, :, idx, :] = blk[l]
        S = C @ (D.reshape(2 * N, 2 * N) @ S)

    # Ms[i, j] = S[j, 2i]; mode-stationary halves Mse[i,m]=Ms[i,2m],
    # Mso[i,m]=Ms[i,2m+1] packed side by side for one DMA.
    Ms = S[:, 0::2].T                                   # [128, 256]
    ms_cat = np.concatenate([Ms[:, 0::2], Ms[:, 1::2]], axis=1)

    s = 2.0 ** np.ceil(np.log2(np.abs(ms_cat).max() / MS_TARGET_MAX))
    s = max(s, 1.0)
    ms_f16 = np.ascontiguousarray(ms_cat / s).astype(np.float16)

    dV = (S ** 2).sum(axis=1)                           # [256]
    bias = ((dV[0::2] + dV[1::2]) / 4.0 - 0.5).astype(np.float32)  # [128]
    return ms_f16, bias, np.float32(s * s)


def make_in_maps(X: np.ndarray, ms_f16: np.ndarray):
    """Per-core input dicts: xt [128, ROWS] f16 (host-transposed), ms."""
    Xt = np.ascontiguousarray(
        X.reshape(N_CORES, ROWS, N_QUMODES).transpose(0, 2, 1).astype(np.float16))
    return [{"xt": Xt[i], "ms": ms_f16} for i in range(N_CORES)]


def postprocess(results, bias: np.ndarray, s2: np.float32) -> np.ndarray:
    """Gather per-core n.T bf16 tiles into the full [BATCH, 128] f32 output,
    undoing the fp16 weight scale and adding the per-mode bias."""
    out = np.empty((BATCH, N_QUMODES), dtype=np.float32)
    for i, r in enumerate(results):
        out[i * ROWS:(i + 1) * ROWS, :] = r["out"].astype(np.float32).T
    out *= s2
    out += bias[None, :]
    return out


def build_bass():
    nc = bacc.Bacc("TRN2", target_bir_lowering=False, debug=False,
                   num_devices=N_CORES)

    xt_d = nc.dram_tensor("xt", [128, ROWS], F16, kind="ExternalInput")
    ms_d = nc.dram_tensor("ms", [128, 256], F16, kind="ExternalInput")
    out_d = nc.dram_tensor("out", [128, ROWS], BF16, kind="ExternalOutput")

    xt_v = xt_d.ap()
    out_v = out_d.ap()

    # Per-sub p-square recipe: 'A' = ACT Square from PSUM + DVE add;
    # 'D' = DVE copy + DVE self-mult + Pool add;
    # 'P' = DVE copy + Pool self-mult + DVE add.
    # ACT is the only engine that can square straight from PSUM (one read);
    # DVE cannot read PSUM twice and Pool cannot read PSUM at all.  The
    # slow Pool ops are kept away from the last subs so the tail drains
    # through the short ACT/DVE chains.
    RECIPE = ['A', 'D', 'P', 'A', 'D', 'P', 'A', 'D', 'P', 'A', 'D', 'P',
              'A', 'D', 'A', 'A']

    with tile.TileContext(nc) as tc:
        with (
            tc.tile_pool(name="const", bufs=1) as const_pool,
            tc.tile_pool(name="xin", bufs=3) as xin_pool,
            tc.tile_pool(name="oout", bufs=3) as oout_pool,
            tc.tile_pool(name="sqx", bufs=3) as sqx_pool,
            tc.tile_pool(name="sqp", bufs=3) as sqp_pool,
            tc.tile_pool(name="cp", bufs=2) as cp_pool,
            tc.tile_pool(name="mux", bufs=2, space="PSUM") as mux_pool,
            tc.tile_pool(name="mup", bufs=2, space="PSUM") as mup_pool,
        ):
            # Tiny ms halves first so the PE can start as soon as x lands.
            mse = const_pool.tile([128, 128], F16)
            nc.sync.dma_start(out=mse, in_=ms_d.ap()[:, 0:128])
            mso = const_pool.tile([128, 128], F16)
            nc.sync.dma_start(out=mso, in_=ms_d.ap()[:, 128:256])

            x_tiles: dict[int, bass.AP] = {}

            def load_chunk(c):
                x_sb = xin_pool.tile([128, CHUNK], F16, tag="x_sb",
                                     name=f"x_sb_{c}")
                if c == 0:
                    # split the first transfer so the PE can start sooner
                    for q in range(4):
                        nc.sync.dma_start(
                            out=x_sb[:, q * MM:(q + 1) * MM],
                            in_=xt_v[:, q * MM:(q + 1) * MM])
                else:
                    nc.sync.dma_start(out=x_sb,
                                      in_=xt_v[:, c * CHUNK:(c + 1) * CHUNK])
                x_tiles[c] = x_sb

            load_chunk(0)
            load_chunk(1)

            for i in range(N_SUBS):
                c, sc = divmod(i, SUBS_PER_CHUNK)
                if sc == 0 and c + 2 < N_CHUNKS:
                    load_chunk(c + 2)
                x_sb = x_tiles[c]

                mu_x = mux_pool.tile([128, SUB], F32, tag="mu")  # 2 PSUM banks
                mu_p = mup_pool.tile([128, SUB], F32, tag="mp")  # 2 PSUM banks
                for q in range(SUB // MM):
                    rhs = x_sb[:, sc * SUB + q * MM: sc * SUB + (q + 1) * MM]
                    nc.tensor.matmul(mu_x[:, q * MM:(q + 1) * MM], mse, rhs,
                                     start=True, stop=True)
                    nc.tensor.matmul(mu_p[:, q * MM:(q + 1) * MM], mso, rhs,
                                     start=True, stop=True)

                sq_x = sqx_pool.tile([128, SUB], BF16, tag="sq_x",
                                     name=f"sq_x_{i}")
                sq_p = sqp_pool.tile([128, SUB], BF16, tag="sq_p",
                                     name=f"sq_p_{i}")
                nc.scalar.activation(sq_x, mu_x,
                                     mybir.ActivationFunctionType.Square)
                recipe = RECIPE[i]
                if recipe == 'A':
                    nc.scalar.activation(sq_p, mu_p,
                                         mybir.ActivationFunctionType.Square)
                    add_eng = nc.vector
                elif recipe == 'D':
                    cp = cp_pool.tile([128, SUB], F32, tag="cp",
                                      name=f"cp_{i}")
                    nc.vector.tensor_copy(cp, mu_p)
                    nc.vector.tensor_tensor(out=sq_p, in0=cp, in1=cp,
                                            op=mybir.AluOpType.mult)
                    add_eng = nc.gpsimd
                else:
                    cp = cp_pool.tile([128, SUB], F32, tag="cp",
                                      name=f"cp_{i}")
                    nc.vector.tensor_copy(cp, mu_p)
                    nc.gpsimd.tensor_tensor(out=sq_p, in0=cp, in1=cp,
                                            op=mybir.AluOpType.mult)
                    add_eng = nc.vector

                o_sb = oout_pool.tile([128, SUB], BF16, tag="o_sb",
                                      name=f"o_sb_{i}")
                add_eng.tensor_tensor(out=o_sb, in0=sq_x, in1=sq_p,
                                      op=mybir.AluOpType.add)
                nc.scalar.dma_start(out=out_v[:, i * SUB:(i + 1) * SUB],
                                    in_=o_sb)
                if sc == SUBS_PER_CHUNK - 1:
                    x_tiles.pop(c, None)

    nc.compile()
    return nc


_NC_CACHE = None


def kernel(**inputs: np.ndarray) -> np.ndarray:
    global _NC_CACHE
    X = np.ascontiguousarray(np.asarray(inputs["inputs"], dtype=np.float32))
    params = np.asarray(inputs["params"], dtype=np.float32)
    assert X.shape == (BATCH, N_QUMODES)

    ms_f16, bias, s2 = host_prep(params)

    if _NC_CACHE is None:
        _NC_CACHE = build_bass()
    nc = _NC_CACHE

    in_maps = make_in_maps(X, ms_f16)
    res = run_bass_kernel_spmd(nc, in_maps, core_ids=list(range(N_CORES)))
    return postprocess(res.results, bias, s2)


# revision 12
# speedup vs baseline: 1.2733x; 1.0704x over previous
"""Trainium2 Bass kernel for the ContinuousVariableQNN problem.

Math reduction (validated against the jax reference on host):
  The reference builds a 256x256 symplectic matrix S from params, then
    mu   = mu0 @ S.T   with mu0[:, 0::2] = 2*inputs (odd cols zero)
    n    = (dsum + mu_x^2 + mu_p^2) / (2*hbar) - 0.5
  Because mu0's p-quadrature entries are all zero, the big matmul collapses to
    mu_dev = inputs @ Ms          with Ms[i, j] = S[j, 2*i]   ([128, 256])
  (factor 2 from displacement and the 1/4 normalization cancel), and
    n[b, m] = mu_dev[b, 2m]^2 + mu_dev[b, 2m+1]^2 + bias[m]
  with bias[m] = (diag(S S^T)[2m] + diag(S S^T)[2m+1])/4 - 0.5 (a constant).

Device strategy (pure data parallelism over 8 cores, batch-sharded):
  The batch is transposed on the HOST so each core receives
  xt [128 features, 16384 batch] -- fully contiguous DMA, no on-chip
  transposes.  Everything runs in float16 on the PE: fp16 streams at
  1 cycle/row (vs fp32r which draws enough power to trip the 0.5-util
  EDPP throttle) and halves input DMA traffic.  fp16's 11 mantissa bits
  survive the ~12x error amplification of this problem (sims at 7.8e-3
  vs the 2e-2 gate; bf16 inputs sim at 2.4e-2 and fail).  Ms overflows
  fp16 range, so the host pre-scales it by a global power of two and
  folds s^2 into the final host-side bias add.

  Mode-stationary matmuls: mu_x.T [128 modes, 512] = Mse.T @ xt chunk,
  ditto mu_p with Mso, PSUM tiles of 1024 (2 banks, bufs=2 -> all 8).
  Squares: ACT Square straight from PSUM (engines may read only ONE
  operand from PSUM, so DVE cannot self-mult PSUM); for 10/16 sub-chunks
  the p-half goes Pool-copy -> DVE self-mult to keep ACT under the DMA
  envelope.  Pair-add on DVE in bf16 (2x mode).  n.T goes back as bf16
  (output traffic halved); per-mode bias lands on the host for free.
"""

import ml_dtypes
import numpy as np

import concourse.bass as bass
import concourse.mybir as mybir
import concourse.tile as tile
from concourse import bacc
from concourse.bass_utils import run_bass_kernel_spmd

N_QUMODES = 128
N_LAYERS = 8
BATCH = 131072
N_CORES = 8
ROWS = BATCH // N_CORES          # 16384 batch columns per core
CHUNK = 2048                     # batch columns per input DMA chunk
N_CHUNKS = ROWS // CHUNK         # 8
SUB = 1024                       # batch columns per compute sub-chunk
SUBS_PER_CHUNK = CHUNK // SUB    # 2
N_SUBS = N_CHUNKS * SUBS_PER_CHUNK
MM = 512                         # matmul free dim (one PSUM bank of fp32)
F32 = mybir.dt.float32
F16 = mybir.dt.float16
BF16 = mybir.dt.bfloat16

# fp16 scaling for Ms (entries up to ~3e5 overflow fp16's 65504 max).
MS_TARGET_MAX = 16384.0


def host_prep(params: np.ndarray):
    """Build fp16 ms [128, 256] = [Mse | Mso]/s, bias [128], and s^2."""
    L, N = N_LAYERS, N_QUMODES
    p = params.reshape(L, N, 3).astype(np.float64)
    th1, r, th2 = p[..., 0], p[..., 1], p[..., 2]

    def rot(th):
        c, s = np.cos(th), np.sin(th)
        return np.stack([np.stack([c, -s], -1), np.stack([s, c], -1)], -2)

    z = np.zeros_like(r)
    sq = np.stack([np.stack([np.exp(-r), z], -1),
                   np.stack([z, np.exp(r)], -1)], -2)
    blk = np.einsum('lnab,lnbc,lncd->lnad', rot(th2), sq, rot(th1))

    t = np.cos(np.pi / 4)
    rr = np.sin(np.pi / 4)
    BS4 = np.array([[t, 0., -rr, 0.],
                    [0., t, 0., -rr],
                    [rr, 0., t, 0.],
                    [0., rr, 0., t]])
    C = np.eye(2 * N)
    for i in range(N - 1):
        C[2 * i:2 * i + 4, :] = BS4 @ C[2 * i:2 * i + 4, :]

    S = np.eye(2 * N)
    idx = np.arange(N)
    for l in range(L):
        D = np.zeros((N, 2, N, 2))
        D[idx, :, idx, :] = blk[l]
        S = C @ (D.reshape(2 * N, 2 * N) @ S)

    # Ms[i, j] = S[j, 2i]; mode-stationary halves Mse[i,m]=Ms[i,2m],
    # Mso[i,m]=Ms[i,2m+1] packed side by side for one DMA.
    Ms = S[:, 0::2].T                                   # [128, 256]
    ms_cat = np.concatenate([Ms[:, 0::2], Ms[:, 1::2]], axis=1)

    s = 2.0 ** np.ceil(np.log2(np.abs(ms_cat).max() / MS_TARGET_MAX))
    s = max(s, 1.0)
    ms_f16 = np.ascontiguousarray(ms_cat / s).astype(np.float16)

    dV = (S ** 2).sum(axis=1)                           # [256]
    bias = ((dV[0::2] + dV[1::2]) / 4.0 - 0.5).astype(np.float32)  # [128]
    return ms_f16, bias, np.float32(s * s)


def make_in_maps(X: np.ndarray, ms_f16: np.ndarray):
    """Per-core input dicts: xt [128, ROWS] f16 (host-transposed), ms."""
    Xt = np.ascontiguousarray(
        X.reshape(N_CORES, ROWS, N_QUMODES).transpose(0, 2, 1).astype(np.float16))
    return [{"xt": Xt[i], "ms": ms_f16} for i in range(N_CORES)]


def postprocess(results, bias: np.ndarray, s2: np.float32) -> np.ndarray:
    """Gather per-core n.T bf16 tiles into the full [BATCH, 128] f32 output,
    undoing the fp16 weight scale and adding the per-mode bias."""
    out = np.empty((BATCH, N_QUMODES), dtype=np.float32)
    for i, r in enumerate(results):
        out[i * ROWS:(i + 1) * ROWS, :] = r["out"].astype(np.float32).T
    out *= s2
    out += bias[None, :]
    return out


def build_bass():
    nc = bacc.Bacc("TRN2", target_bir_lowering=False, debug=False,
                   num_devices=N_CORES)

    xt_d = nc.dram_tensor("xt", [128, ROWS], F16, kind="ExternalInput")
    ms_d = nc.dram_tensor("ms", [128, 256], F16, kind="ExternalInput")
    out_d = nc.dram_tensor("out", [128, ROWS], BF16, kind="ExternalOutput")

    xt_v = xt_d.ap()
    out_v = out_d.ap()

    # Per-sub p-square recipe: 'A' = ACT Square from PSUM + DVE add;
    # 'D' = DVE copy + DVE self-mult + Pool add;
    # 'P' = DVE copy + Pool self-mult + DVE add.
    # ACT is the only engine that can square straight from PSUM (one read);
    # DVE cannot read PSUM twice and Pool cannot read PSUM at all.  The
    # slow Pool ops are kept away from the last subs so the tail drains
    # through the short ACT/DVE chains.
    RECIPE = ['A', 'D', 'P', 'A', 'D', 'P', 'A', 'D', 'P', 'A', 'D', 'P',
              'A', 'D', 'A', 'A']

    with tile.TileContext(nc) as tc:
        with (
            tc.tile_pool(name="const", bufs=1) as const_pool,
            tc.tile_pool(name="xin", bufs=3) as xin_pool,
            tc.tile_pool(name="oout", bufs=3) as oout_pool,
            tc.tile_pool(name="sqx", bufs=3) as sqx_pool,
            tc.tile_pool(name="sqp", bufs=3) as sqp_pool,
            tc.tile_pool(name="cp", bufs=2) as cp_pool,
            tc.tile_pool(name="mux", bufs=2, space="PSUM") as mux_pool,
            tc.tile_pool(name="mup", bufs=2, space="PSUM") as mup_pool,
        ):
            # Tiny ms halves first so the PE can start as soon as x lands.
            mse = const_pool.tile([128, 128], F16)
            nc.sync.dma_start(out=mse, in_=ms_d.ap()[:, 0:128])
            mso = const_pool.tile([128, 128], F16)
            nc.sync.dma_start(out=mso, in_=ms_d.ap()[:, 128:256])

            x_tiles: dict[int, bass.AP] = {}

            def load_chunk(c):
                x_sb = xin_pool.tile([128, CHUNK], F16, tag="x_sb",
                                     name=f"x_sb_{c}")
                if c == 0:
                    # split the first transfer so the PE can start sooner
                    for q in range(4):
                        nc.sync.dma_start(
                            out=x_sb[:, q * MM:(q + 1) * MM],
                            in_=xt_v[:, q * MM:(q + 1) * MM])
                else:
                    nc.sync.dma_start(out=x_sb,
                                      in_=xt_v[:, c * CHUNK:(c + 1) * CHUNK])
                x_tiles[c] = x_sb

            load_chunk(0)
            load_chunk(1)

            for i in range(N_SUBS):
                c, sc = divmod(i, SUBS_PER_CHUNK)
                if sc == 0 and c + 2 < N_CHUNKS:
                    load_chunk(c + 2)
                x_sb = x_tiles[c]

                mu_x = mux_pool.tile([128, SUB], F32, tag="mu")  # 2 PSUM banks
                mu_p = mup_pool.tile([128, SUB], F32, tag="mp")  # 2 PSUM banks
                for q in range(SUB // MM):
                    rhs = x_sb[:, sc * SUB + q * MM: sc * SUB + (q + 1) * MM]
                    nc.tensor.matmul(mu_x[:, q * MM:(q + 1) * MM], mse, rhs,
                                     start=True, stop=True)
                    nc.tensor.matmul(mu_p[:, q * MM:(q + 1) * MM], mso, rhs,
                                     start=True, stop=True)

                sq_x = sqx_pool.tile([128, SUB], BF16, tag="sq_x",
                                     name=f"sq_x_{i}")
                sq_p = sqp_pool.tile([128, SUB], BF16, tag="sq_p",
                                     name=f"sq_p_{i}")
                nc.scalar.activation(sq_x, mu_x,
                                     mybir.ActivationFunctionType.Square)
                recipe = RECIPE[i]
                if recipe == 'A':
                    nc.scalar.activation(sq_p, mu_p,
                                         mybir.ActivationFunctionType.Square)
                    add_eng = nc.vector
                elif recipe == 'D':
                    cp = cp_pool.tile([128, SUB], F32, tag="cp",
                                      name=f"cp_{i}")
                    nc.vector.tensor_copy(cp, mu_p)
                    nc.vector.tensor_tensor(out=sq_p, in0=cp, in1=cp,
                                            op=mybir.AluOpType.mult)
                    add_eng = nc.gpsimd
                else:
                    cp = cp_pool.tile([128, SUB], F32, tag="cp",
                                      name=f"cp_{i}")
                    nc.vector.tensor_copy(cp, mu_p)
                    nc.gpsimd.tensor_tensor(out=sq_p, in0=cp, in1=cp,
                                            op=mybir.AluOpType.mult)
                    add_eng = nc.vector

                o_sb = oout_pool.tile([128, SUB], BF16, tag="o_sb",
                                      name=f"o_sb_{i}")
                add_eng.tensor_tensor(out=o_sb, in0=sq_x, in1=sq_p,
                                      op=mybir.AluOpType.add)
                nc.sync.dma_start(out=out_v[:, i * SUB:(i + 1) * SUB],
                                    in_=o_sb)
                if sc == SUBS_PER_CHUNK - 1:
                    x_tiles.pop(c, None)

    nc.compile()
    return nc


_NC_CACHE = None


def kernel(**inputs: np.ndarray) -> np.ndarray:
    global _NC_CACHE
    X = np.ascontiguousarray(np.asarray(inputs["inputs"], dtype=np.float32))
    params = np.asarray(inputs["params"], dtype=np.float32)
    assert X.shape == (BATCH, N_QUMODES)

    ms_f16, bias, s2 = host_prep(params)

    if _NC_CACHE is None:
        _NC_CACHE = build_bass()
    nc = _NC_CACHE

    in_maps = make_in_maps(X, ms_f16)
    res = run_bass_kernel_spmd(nc, in_maps, core_ids=list(range(N_CORES)))
    return postprocess(res.results, bias, s2)
